# revision 2
# baseline (speedup 1.0000x reference)
"""VPT-style transformer block kernel for TRN2, 8-core data-parallel.

Token order per batch is permuted to PCP = [prompts(32), cls(1), patch(196)];
attention is permutation-equivariant under a consistent permutation of q/k/v +
mask, so we only un-permute at the final output DMA.

Per-core layouts:
  xT      : b-major channel-major f32  [8 ptiles][128, 1832]
  xn1,xn2 : group-major bf16 (pc-block cols 0..1575, pr-block 1576..1831)
  qk_bf   : b-major bf16 [16 ptiles][128, 1832]  (q tiles 0-7, k tiles 8-15)
  vT_aug  : per (b, kchunk) bf16 [128, 16*65]  (65th col per head = 1.0 -> Z)
  o_bf    : group-major bf16
  x1      : group-major f32 (spilled to DRAM, reloaded in MLP phase)
  y       : group-major f32 -> PE-transpose -> token-major -> DMA out
"""

import numpy as np
import ml_dtypes

import concourse.bass as bass
import concourse.mybir as mybir
import concourse.tile as tile
from concourse import bacc
from concourse.masks import make_identity

F32 = mybir.dt.float32
F32R = mybir.dt.float32r
BF16 = mybir.dt.bfloat16
AF = mybir.ActivationFunctionType

B, N, C, H, O, P = 64, 229, 1024, 16, 32, 196
D = C // H
SCALE = D ** -0.5
EPS = 1e-5
HID = 4 * C
NCORES = 8
BL = B // NCORES      # 8
PC = 1 + P            # 197
PR = O                # 32
TT = BL * N           # 1832
NPC = BL * PC         # 1576
NPR = BL * PR         # 256
CT = C // 128         # 8
HT = HID // 128       # 32

PC_CHUNKS = [(0, 512), (512, 512), (1024, 512), (1536, NPC - 1536)]
ALL_CHUNKS = PC_CHUNKS + [(NPC, NPR)]

DEBUG_TAPS = False
PHASES = 99
KC0 = (0, 128)      # PCP tokens 0..127   (pr 0..31 + pc 0..95)
KC1 = (101, 128)    # PCP tokens 101..228 (pc 69..196); rows 0..26 dup-zeroed


def _bf(x):
    return np.asarray(x, dtype=ml_dtypes.bfloat16)


def prep_weights(i):
    """Host-side: fold LN gains/biases into weights, cast to bf16."""
    i = {k: np.asarray(v, np.float32) for k, v in i.items()}
    w = {}
    for tag, wqkv, bqkv, g, b in (
        ("pc", i["w_qkv"], i["b_qkv"], i["n1_g"], i["n1_b"]),
        ("pr", i["w_qkv_p"], i["b_qkv_p"], i["n1p_g"], i["n1p_b"]),
    ):
        weff = wqkv * g[:, None]
        beff = bqkv + b @ wqkv
        wqk = weff[:, : 2 * C]
        w[f"wqk_{tag}"] = _bf(np.ascontiguousarray(
            wqk.reshape(CT, 128, 16, 128).transpose(2, 0, 1, 3)))
        w[f"bqk_{tag}"] = np.ascontiguousarray(beff[: 2 * C]).astype(np.float32)
        w[f"wv_{tag}"] = _bf(np.ascontiguousarray(weff[:, 2 * C:]))
        w[f"bv_{tag}"] = np.ascontiguousarray(beff[2 * C:]).astype(np.float32)

    for tag, wp, bp in (("pc", i["w_proj"], i["b_proj"]),
                        ("pr", i["w_proj_p"], i["b_proj_p"])):
        w[f"wproj_{tag}"] = _bf(np.ascontiguousarray(
            wp.reshape(CT, 128, CT, 128).transpose(2, 0, 1, 3)))
        w[f"bproj_{tag}"] = np.asarray(bp, np.float32)

    for tag, f1w, f1b, f2w, f2b, g, b in (
        ("pc", i["fc1_w"], i["fc1_b"], i["fc2_w"], i["fc2_b"], i["n2_g"], i["n2_b"]),
        ("pr", i["pfc1_w"], i["pfc1_b"], i["pfc2_w"], i["pfc2_b"], i["n2p_g"], i["n2p_b"]),
    ):
        f1eff = f1w * g[:, None]
        f1beff = f1b + b @ f1w
        w[f"wf1_{tag}"] = _bf(np.ascontiguousarray(
            f1eff.reshape(CT, 128, HT, 128).transpose(2, 0, 1, 3)))
        w[f"bf1_{tag}"] = np.asarray(f1beff, np.float32)
        w[f"wf2_{tag}"] = _bf(np.ascontiguousarray(
            f2w.reshape(HT, 128, CT, 128).transpose(2, 0, 1, 3)))
        w[f"bf2_{tag}"] = np.asarray(f2b, np.float32)
    return w


PERM = np.concatenate([np.arange(1, 33), [0], np.arange(33, 229)])


def prep_x(x):
    xp = x[:, PERM, :]
    xp = xp.reshape(NCORES, BL * N, C)
    return [np.ascontiguousarray(xp[c].T).astype(np.float32) for c in range(NCORES)]


def unpermute_out(y):
    inv = np.empty(N, np.int64)
    inv[PERM] = np.arange(N)
    return y[:, inv, :]



class _Pool:
    """tile_pool wrapper with explicit close()."""
    def __init__(self, tc, **kw):
        self._cm = tc.tile_pool(**kw)
        self._p = self._cm.__enter__()

    def tile(self, *a, **k):
        if "name" not in k:
            k["name"] = k.get("tag") or "t"
        return self._p.tile(*a, **k)

    def close(self):
        self._cm.__exit__(None, None, None)


def build_program(nc):
    def din(name, shape, dt):
        return nc.dram_tensor(name, list(shape), dt, kind="ExternalInput").ap()

    d = {}
    d["xT"] = din("xT", (C, TT), F32)
    for t in ("pc", "pr"):
        d[f"wqk_{t}"] = din(f"wqk_{t}", (16, CT, 128, 128), BF16)
        d[f"bqk_{t}"] = din(f"bqk_{t}", (2 * C,), F32)
        d[f"wv_{t}"] = din(f"wv_{t}", (C, C), BF16)
        d[f"bv_{t}"] = din(f"bv_{t}", (C,), F32)
        d[f"wproj_{t}"] = din(f"wproj_{t}", (CT, CT, 128, 128), BF16)
        d[f"bproj_{t}"] = din(f"bproj_{t}", (C,), F32)
        d[f"wf1_{t}"] = din(f"wf1_{t}", (HT, CT, 128, 128), BF16)
        d[f"bf1_{t}"] = din(f"bf1_{t}", (HID,), F32)
        d[f"wf2_{t}"] = din(f"wf2_{t}", (CT, HT, 128, 128), BF16)
        d[f"bf2_{t}"] = din(f"bf2_{t}", (C,), F32)
    d["out"] = nc.dram_tensor("out", [C, TT], F32, kind="ExternalOutput").ap()
    with tile.TileContext(nc) as tc:
        _emit(tc, nc, d)




def _sce_recip(nc, out, in_):
    """ScalarE LUT reciprocal. The bass wrapper refuses Reciprocal for
    accuracy reasons; for softmax denominators / LN rstd the ~1e-3 LUT error
    is far below the bf16 noise floor, and DVE reciprocal is ~6.5 cyc/elem."""
    eng = nc.scalar
    return eng.add_instruction(
        mybir.InstActivation(
            name=nc.get_next_instruction_name(),
            func=AF.Reciprocal,
            ins=[eng.lower_ap(in_),
                 mybir.ImmediateValue(dtype=F32, value=0.0),
                 mybir.ImmediateValue(dtype=F32, value=1.0),
                 mybir.ImmediateValue(dtype=F32, value=0.0)],
            outs=[eng.lower_ap(out)],
        ))


def _pcap(p):
    return {0: 128, 32: 32, 64: 64, 96: 32}[p]


def _psplit2(dst0, src0, nrows):
    """Split so BOTH dst and src partition slices are engine-legal.
    Yields (dst_start, src_start, count)."""
    out = []
    done = 0
    while done < nrows:
        a, b = dst0 + done, src0 + done
        n = min(_pcap(a), _pcap(b), nrows - done)
        out.append((a, b, n))
        done += n
    return out


def _ln_rows(nc, tc, consts, x_tiles, tag):
    """LN stats over channel dim (partitions). Opens bc pool FIRST (returned;
    caller closes). Returns (rb, mrb, bc_pool): [128, TT] f32 bcast rows."""
    bc_pool = _Pool(tc, name=f"bc_{tag}", bufs=1)
    rows = _Pool(tc, name=f"rows_{tag}", bufs=1)
    ps_pool = _Pool(tc, name=f"lnps_{tag}", bufs=2, space="PSUM")
    sq_pool = _Pool(tc, name=f"lnsq_{tag}", bufs=2)

    m_row = rows.tile([1, TT], F32, tag="m")
    q_row = rows.tile([1, TT], F32, tag="q")
    ones_bf, ones1_bf, eps_t = consts

    CH = 512
    for ci in range((TT + CH - 1) // CH):
        c0, cw = ci * CH, min(CH, TT - ci * CH)
        ps = ps_pool.tile([1, CH], F32, tag="s")
        pq = ps_pool.tile([1, CH], F32, tag="q")
        for ct in range(CT):
            xs = x_tiles[ct][:, c0:c0 + cw]
            xbf = sq_pool.tile([128, CH], BF16, tag="xbf")
            nc.vector.tensor_copy(xbf[:, :cw], xs)
            nc.tensor.matmul(ps[:, :cw], ones_bf, xbf[:, :cw],
                             start=(ct == 0), stop=(ct == CT - 1))
            xsq = sq_pool.tile([128, CH], BF16, tag="xsq")
            nc.vector.tensor_mul(xsq[:, :cw], xs, xs)
            nc.tensor.matmul(pq[:, :cw], ones_bf, xsq[:, :cw],
                             start=(ct == 0), stop=(ct == CT - 1))
        nc.scalar.activation(m_row[:, c0:c0 + cw], ps[:, :cw], AF.Copy,
                             scale=1.0 / C)
        nc.scalar.activation(q_row[:, c0:c0 + cw], pq[:, :cw], AF.Copy,
                             scale=1.0 / C)

    r_row = rows.tile([1, TT], F32, tag="r")
    nc.vector.tensor_mul(r_row, m_row, m_row)
    nc.vector.tensor_sub(q_row, q_row, r_row)       # q <- var
    nc.scalar.activation(q_row, q_row, AF.Sqrt, bias=eps_t)
    _sce_recip(nc, r_row, q_row)                    # r <- rstd
    mr_row = m_row
    nc.vector.tensor_mul(mr_row, m_row, r_row)      # m <- m*rstd

    r_bf = rows.tile([1, TT], BF16, tag="rbf")
    mr_bf = rows.tile([1, TT], BF16, tag="mrbf")
    nc.vector.tensor_copy(r_bf, r_row)
    nc.vector.tensor_copy(mr_bf, mr_row)
    rb = bc_pool.tile([128, TT], F32, tag="rb")
    mrb = bc_pool.tile([128, TT], F32, tag="mrb")
    BCH = 458
    for ci in range((TT + BCH - 1) // BCH):
        c0, cw = ci * BCH, min(BCH, TT - ci * BCH)
        for src, dst in ((r_bf, rb), (mr_bf, mrb)):
            pb = ps_pool.tile([128, BCH], F32, tag="bc")
            nc.tensor.matmul(pb[:, :cw], ones1_bf,
                             src[:, c0:c0 + cw],
                             start=True, stop=True)
            nc.vector.tensor_copy(dst[:, c0:c0 + cw], pb[:, :cw])
    sq_pool.close()
    ps_pool.close()
    rows.close()
    return rb, mrb, bc_pool


def _apply_ln(nc, xa, rb, mrb, xn):
    """xn (group-major bf16) = (x - m)*r from b-major x. Two TT passes:
    pass1 writes x*r scattered to group-major; pass2 subtracts m*r in place."""
    for ct in range(CT):
        src = xa[ct].rearrange("p (b n) -> p b n", b=BL)
        mv = mrb.rearrange("p (b n) -> p b n", b=BL)
        rv = rb.rearrange("p (b n) -> p b n", b=BL)
        o = xn[ct]
        opc = o[:, :NPC].rearrange("p (b n) -> p b n", n=PC)
        opr = o[:, NPC:].rearrange("p (b n) -> p b n", n=PR)
        nc.vector.tensor_mul(opc, src[:, :, PR:], rv[:, :, PR:])
        nc.vector.tensor_mul(opr, src[:, :, :PR], rv[:, :, :PR])
        nc.vector.tensor_sub(opc, opc, mv[:, :, PR:])
        nc.vector.tensor_sub(opr, opr, mv[:, :, :PR])


def _emit(tc, nc, d):
    const = _Pool(tc, name="const", bufs=1)
    ones_bf = const.tile([128, 1], BF16, tag="ones128")
    nc.vector.memset(ones_bf, 1.0)
    ones1_bf = const.tile([1, 128], BF16, tag="ones1x128")
    nc.vector.memset(ones1_bf, 1.0)
    eps_t = const.tile([1, 1], F32, tag="eps")
    nc.vector.memset(eps_t, EPS)
    ones64 = const.tile([1, 64], BF16, tag="ones1x64")
    nc.vector.memset(ones64, 1.0)
    consts = (ones_bf, ones1_bf, eps_t)

    # x arena: x, overwritten in place by x1 = x + attn_out at proj time
    xa_pool = _Pool(tc, name="xarena", bufs=1)
    xa = [xa_pool.tile([128, TT], F32, tag=f"x{ct}") for ct in range(CT)]
    for ct in range(CT):
        nc.sync.dma_start(out=xa[ct], in_=d["xT"][128 * ct:128 * (ct + 1), :])

    p_vT = _Pool(tc, name="p_vT", bufs=1)
    vT = [[p_vT.tile([128, 16 * 64], BF16, tag=f"vT{b}_{kc}")
           for kc in range(2)] for b in range(BL)]
    onesblk = const.tile([128, 64], BF16, tag="onesblk")
    nc.vector.memset(onesblk, 1.0)
    p_opr = _Pool(tc, name="p_opr", bufs=1)
    o_pr = [p_opr.tile([128, NPR], BF16, tag=f"opr{i}") for i in range(CT)]
    p_obp = _Pool(tc, name="p_obp", bufs=1)
    obp = [[p_obp.tile([128, 2 * PC], BF16, tag=f"obp{bp}_{i}")
            for i in range(CT)] for bp in range(BL // 2)]
    p_xn1 = _Pool(tc, name="p_xn1", bufs=1)
    xn1 = [p_xn1.tile([128, TT], BF16, tag=f"xn1_{ct}") for ct in range(CT)]

    # ---------------- LN1 ----------------
    with nc.named_scope("ln1"):
        rb1, mrb1, bc1 = _ln_rows(nc, tc, consts, xa, "ln1")
        _apply_ln(nc, xa, rb1, mrb1, xn1)
        bc1.close()
    if PHASES < 2:
        for p in (p_xn1, p_obp, p_opr, p_vT, xa_pool, const):
            p.close()
        return

    # ---------------- v (transposed, ones-augmented) ----------------
    with nc.named_scope("vmm"):
        bvb = {}
        p_bvb = _Pool(tc, name="p_bvb", bufs=1)
        p_bvrow = _Pool(tc, name="p_bvrow", bufs=1)
        ps_bc = _Pool(tc, name="vbc_ps", bufs=2, space="PSUM")
        for t in ("pc", "pr"):
            brow = p_bvrow.tile([1, C], F32, tag=f"bvrow_{t}")
            nc.sync.dma_start(out=brow,
                              in_=d[f"bv_{t}"].rearrange("(o c) -> o c", o=1))
            brow_bf = p_bvrow.tile([1, C], BF16, tag=f"bvrowbf_{t}")
            nc.vector.tensor_copy(brow_bf, brow)
            bvb[t] = p_bvb.tile([128, C], F32, tag=f"bvb_{t}")
            for half in range(2):
                pb = ps_bc.tile([128, 512], F32, tag="bc")
                nc.tensor.matmul(pb, ones1_bf,
                                 brow_bf[:, 512 * half:512 * (half + 1)],
                                 start=True, stop=True)
                nc.vector.tensor_copy(bvb[t][:, 512 * half:512 * (half + 1)], pb)
        ps_bc.close()
        p_bvrow.close()

        # one weight set resident at a time
        for sname, groups in (
            ("pc", [(b, kc, row0, nrows, sc0)
                    for b in range(BL)
                    for kc, row0, nrows, sc0 in
                    ((0, PR, 96, PC * b), (1, 0, 128, PC * b + 69))]),
            ("pr", [(b, 0, 0, PR, NPC + PR * b) for b in range(BL)]),
        ):
            p_wv = _Pool(tc, name=f"p_wv_{sname}", bufs=1)
            wv_sb = [p_wv.tile([128, C], BF16, tag=f"wv{ci}")
                     for ci in range(CT)]
            for ci in range(CT):
                nc.sync.dma_start(out=wv_sb[ci],
                                  in_=d[f"wv_{sname}"][128 * ci:128 * (ci + 1), :])
            ps_v = _Pool(tc, name=f"v_ps_{sname}", bufs=3, space="PSUM")
            for b, kc, row0, nrows, sc0 in groups:
                pv = ps_v.tile([128, C], F32, tag="v")
                for ci in range(CT):
                    lhs = xn1[ci][:, sc0:sc0 + nrows]
                    for half in range(2):
                        nc.tensor.matmul(
                            pv[:nrows, 512 * half:512 * (half + 1)],
                            lhs,
                            wv_sb[ci][:, 512 * half:512 * (half + 1)],
                            start=(ci == 0), stop=(ci == CT - 1))
                # engine partition rule: base in {0,32,64,96}; <=32 from
                # 32/96, <=64 from 64, <=128 from 0, on BOTH src and dst
                for d0, s0, sn in _psplit2(row0, 0, nrows):
                    nc.vector.tensor_add(
                        vT[b][kc][d0:d0 + sn, :],
                        pv[s0:s0 + sn, :],
                        bvb[sname][d0:d0 + sn, :])
            ps_v.close()
            p_wv.close()
        p_bvb.close()
    if PHASES < 3:
        for p in (p_xn1, p_obp, p_opr, p_vT, xa_pool, const):
            p.close()
        return

    # ---------------- qk + attention, per head-pair ----------------
    with nc.named_scope("attn"):
        bqk_sb = {}
        for t in ("pc", "pr"):
            bt = const.tile([128, 16], F32, tag=f"bqk_{t}")
            nc.sync.dma_start(
                out=bt, in_=d[f"bqk_{t}"].rearrange("(a p) -> p a", p=128))
            bqk_sb[t] = bt
        qk_pool = _Pool(tc, name="qk", bufs=2)
        wq_pool = _Pool(tc, name="wqk", bufs=2)
        ps_qk = _Pool(tc, name="qk_ps", bufs=2, space="PSUM")
        epool = _Pool(tc, name="attn_e", bufs=6)
        zpool = _Pool(tc, name="attn_z", bufs=4)
        ps_sT = _Pool(tc, name="sT_ps", bufs=2, space="PSUM")
        ps_o = _Pool(tc, name="o_ps", bufs=2, space="PSUM")
        ps_z = _Pool(tc, name="z_ps", bufs=2, space="PSUM")

        for hp in range(8):
            qk_t = {}
            for qk_kind, co in (("q", hp), ("k", 8 + hp)):
                tl = qk_pool.tile([128, TT], BF16, tag=qk_kind)
                qk_t[qk_kind] = tl
                w_sb = {}
                for t in ("pc", "pr"):
                    w_sb[t] = wq_pool.tile([128, CT * 128], BF16, tag=f"w_{t}")
                    nc.sync.dma_start(
                        out=w_sb[t].rearrange("p (c e) -> p c e", c=CT),
                        in_=d[f"wqk_{t}"][co].rearrange("c p e -> p c e"))
                for ci_ch, (c0, cw) in enumerate(ALL_CHUNKS):
                    sname = "pr" if ci_ch == 4 else "pc"
                    pt = ps_qk.tile([128, 512], F32, tag="qk")
                    for ci in range(CT):
                        nc.tensor.matmul(
                            pt[:, :cw], w_sb[sname][:, 128 * ci:128 * (ci + 1)],
                            xn1[ci][:, c0:c0 + cw],
                            start=(ci == 0), stop=(ci == CT - 1))
                    bias_ap = bqk_sb[sname][:, co:co + 1]
                    if sname == "pr":
                        dst = tl.rearrange("p (b n) -> p b n", n=N)[:, :, 0:PR]
                        nc.scalar.activation(
                            dst, pt[:, :cw].rearrange("p (b n) -> p b n", n=PR),
                            AF.Identity, bias=bias_ap)
                    else:
                        g = c0
                        while g < c0 + cw:
                            b = g // PC
                            p0 = g % PC
                            run = min(PC - p0, c0 + cw - g)
                            dst = tl[:, N * b + PR + p0: N * b + PR + p0 + run]
                            nc.scalar.activation(dst, pt[:, g - c0:g - c0 + run],
                                                 AF.Identity, bias=bias_ap)
                            g += run
            for bp in range(BL // 2):
                b0 = 2 * bp
                for h in (2 * hp, 2 * hp + 1):
                    r0 = 64 * (h % 2)
                    q_ap = qk_t["q"][r0:r0 + 64, N * b0:N * (b0 + 2)]
                    es = []
                    for kc, (t0, tw) in enumerate((KC0, KC1)):
                        e = epool.tile([128, 2 * N], BF16, tag="e")
                        for j in range(2):
                            ps = ps_sT.tile([128, 2 * N], F32, tag="sT")
                            k_ap = qk_t["k"][
                                r0:r0 + 64,
                                N * (b0 + j) + t0: N * (b0 + j) + t0 + tw]
                            nc.tensor.matmul(ps[:tw, N * j:N * (j + 1)], k_ap,
                                             q_ap[:, N * j:N * (j + 1)],
                                             start=True, stop=True)
                            nc.scalar.activation(e[:tw, N * j:N * (j + 1)],
                                                 ps[:tw, N * j:N * (j + 1)],
                                                 AF.Exp, scale=SCALE)
                        if kc == 0:
                            ev = e.rearrange("p (b n) -> p b n", b=2)
                            nc.vector.memset(ev[0:PR, :, PR:], 0.0)
                        else:
                            nc.vector.memset(e[0:27, :], 0.0)
                        es.append(e)
                    po = ps_o.tile([64, 2 * N], F32, tag="o")
                    po_z = ps_z.tile([64, 2 * N], F32, tag="z")
                    for j in range(2):
                        for kc in range(2):
                            nc.tensor.matmul(
                                po[:, N * j:N * (j + 1)],
                                vT[b0 + j][kc][:, 64 * h:64 * (h + 1)],
                                es[kc][:, N * j:N * (j + 1)],
                                start=(kc == 0), stop=(kc == 1))
                            nc.tensor.matmul(
                                po_z[:, N * j:N * (j + 1)],
                                onesblk,
                                es[kc][:, N * j:N * (j + 1)],
                                start=(kc == 0), stop=(kc == 1))
                    zb = zpool.tile([64, 2 * N], F32, tag="zb")
                    _sce_recip(nc, zb, po_z)
                    po_v = po.rearrange("p (b n) -> p b n", b=2)
                    zb_v = zb.rearrange("p (b n) -> p b n", b=2)
                    nc.vector.tensor_mul(
                        obp[bp][hp][r0:r0 + 64, :].rearrange(
                            "p (b n) -> p b n", b=2),
                        po_v[:, :, PR:], zb_v[:, :, PR:])
                    nc.vector.tensor_mul(
                        o_pr[hp][r0:r0 + 64, PR * b0:PR * (b0 + 2)].rearrange(
                            "p (b n) -> p b n", b=2),
                        po_v[:, :, :PR], zb_v[:, :, :PR])
        for p in (ps_z, ps_o, ps_sT, ps_qk, zpool, epool, wq_pool, qk_pool):
            p.close()
    p_xn1.close()
    if PHASES < 4:
        for p in (p_obp, p_opr, p_vT, xa_pool, const):
            p.close()
        return

    # ---------------- pc-proj (+residual in place) ----------------
    bproj_sb = {}
    for t in ("pc", "pr"):
        bt = const.tile([128, CT], F32, tag=f"bproj_{t}")
        nc.sync.dma_start(
            out=bt, in_=d[f"bproj_{t}"].rearrange("(a p) -> p a", p=128))
        bproj_sb[t] = bt
    with nc.named_scope("proj"):
        wp_pool = _Pool(tc, name="wproj", bufs=2)
        pj_tmp = _Pool(tc, name="pj_tmp", bufs=3)
        ps_pj = _Pool(tc, name="pj_ps", bufs=2, space="PSUM")
        for bp in range(BL // 2):
            b0 = 2 * bp
            for co in range(CT):
                w_sb = wp_pool.tile([128, CT * 128], BF16, tag="w")
                nc.sync.dma_start(
                    out=w_sb.rearrange("p (c e) -> p c e", c=CT),
                    in_=d["wproj_pc"][co].rearrange("c p e -> p c e"))
                pt = ps_pj.tile([128, 2 * PC], F32, tag="pj")
                for ci in range(CT):
                    nc.tensor.matmul(pt, w_sb[:, 128 * ci:128 * (ci + 1)],
                                     obp[bp][ci],
                                     start=(ci == 0), stop=(ci == CT - 1))
                tmp = pj_tmp.tile([128, 2 * PC], F32, tag="t")
                nc.vector.tensor_scalar_add(tmp, pt, bproj_sb["pc"][:, co:co + 1])
                for j in range(2):
                    xcols = xa[co][:, N * (b0 + j) + PR:N * (b0 + j + 1)]
                    nc.vector.tensor_add(xcols, tmp[:, PC * j:PC * (j + 1)],
                                         xcols)
        ps_pj.close(); pj_tmp.close(); wp_pool.close()
    p_obp.close()

    # ---------------- pr-proj ----------------
    with nc.named_scope("prproj"):
        wp_pool = _Pool(tc, name="wprojpr", bufs=2)
        pj_tmp = _Pool(tc, name="prtmp", bufs=2)
        ps_pj = _Pool(tc, name="prpj_ps", bufs=2, space="PSUM")
        for co in range(CT):
            w_sb = wp_pool.tile([128, CT * 128], BF16, tag="w")
            nc.sync.dma_start(
                out=w_sb.rearrange("p (c e) -> p c e", c=CT),
                in_=d["wproj_pr"][co].rearrange("c p e -> p c e"))
            pt = ps_pj.tile([128, NPR], F32, tag="pj")
            for ci in range(CT):
                nc.tensor.matmul(pt, w_sb[:, 128 * ci:128 * (ci + 1)], o_pr[ci],
                                 start=(ci == 0), stop=(ci == CT - 1))
            tmp = pj_tmp.tile([128, NPR], F32, tag="t")
            nc.vector.tensor_scalar_add(tmp, pt, bproj_sb["pr"][:, co:co + 1])
            xv = xa[co].rearrange("p (b n) -> p b n", n=N)[:, :, 0:PR]
            nc.vector.tensor_add(xv, tmp.rearrange("p (b n) -> p b n", n=PR), xv)
        ps_pj.close(); pj_tmp.close(); wp_pool.close()
    p_opr.close()
    p_vT.close()
    if PHASES < 5:
        for p in (xa_pool, const):
            p.close()
        return

    # ---------------- LN2 ----------------
    p_xn2 = _Pool(tc, name="p_xn2", bufs=1)
    xn2 = [p_xn2.tile([128, TT], BF16, tag=f"xn2_{ct}") for ct in range(CT)]
    with nc.named_scope("ln2"):
        rb2, mrb2, bc2 = _ln_rows(nc, tc, consts, xa, "ln2")
        _apply_ln(nc, xa, rb2, mrb2, xn2)
        bc2.close()
    if PHASES < 6:
        for p in (p_xn2, xa_pool, const):
            p.close()
        return

    # ---------------- MLP + output ----------------
    with nc.named_scope("mlp"):
        bsb = {}
        for t in ("pc", "pr"):
            bt = const.tile([128, HT], F32, tag=f"bf1_{t}")
            nc.sync.dma_start(
                out=bt, in_=d[f"bf1_{t}"].rearrange("(a p) -> p a", p=128))
            bsb[f"f1_{t}"] = bt
            bt2 = const.tile([128, CT], F32, tag=f"bf2_{t}")
            nc.sync.dma_start(
                out=bt2, in_=d[f"bf2_{t}"].rearrange("(a p) -> p a", p=128))
            bsb[f"f2_{t}"] = bt2

        hpool = _Pool(tc, name="h", bufs=1)
        w1pool = _Pool(tc, name="wf1", bufs=3)
        w2pool = _Pool(tc, name="wf2", bufs=3)
        ypool = _Pool(tc, name="y", bufs=3)
        ps_f1 = _Pool(tc, name="f1_ps", bufs=2, space="PSUM")
        ps_f2 = _Pool(tc, name="f2_ps", bufs=2, space="PSUM")

        for sname in ("pc", "pr"):
            chunks = PC_CHUNKS if sname == "pc" else [(NPC, NPR)]
            for (c0, cw) in chunks:
                hs = []
                for hc in range(HT):
                    w1 = w1pool.tile([128, CT * 128], BF16, tag="w1")
                    nc.sync.dma_start(
                        out=w1.rearrange("p (c e) -> p c e", c=CT),
                        in_=d[f"wf1_{sname}"][hc].rearrange("c p e -> p c e"))
                    ph = ps_f1.tile([128, 512], F32, tag="f1")
                    for ci in range(CT):
                        nc.tensor.matmul(
                            ph[:, :cw], w1[:, 128 * ci:128 * (ci + 1)],
                            xn2[ci][:, c0:c0 + cw],
                            start=(ci == 0), stop=(ci == CT - 1))
                    hsb = hpool.tile([128, 512], BF16, tag=f"h{hc}")
                    nc.scalar.activation(hsb[:, :cw], ph[:, :cw], AF.Gelu,
                                         bias=bsb[f"f1_{sname}"][:, hc:hc + 1])
                    hs.append(hsb)
                for co in range(CT):
                    w2 = w2pool.tile([128, HT * 128], BF16, tag="w2")
                    nc.sync.dma_start(
                        out=w2.rearrange("p (c e) -> p c e", c=HT),
                        in_=d[f"wf2_{sname}"][co].rearrange("c p e -> p c e"))
                    py = ps_f2.tile([128, 512], F32, tag="f2")
                    for hc in range(HT):
                        nc.tensor.matmul(
                            py[:, :cw], w2[:, 128 * hc:128 * (hc + 1)],
                            hs[hc][:, :cw],
                            start=(hc == 0), stop=(hc == HT - 1))
                    tmp = ypool.tile([128, 512], F32, tag="f2t")
                    nc.vector.tensor_scalar_add(tmp[:, :cw], py[:, :cw],
                                                bsb[f"f2_{sname}"][:, co:co + 1])
                    yt = ypool.tile([128, 512], F32, tag="y")
                    if sname == "pr":
                        nc.vector.tensor_add(
                            yt[:, :cw].rearrange("p (b n) -> p b n", n=PR),
                            tmp[:, :cw].rearrange("p (b n) -> p b n", n=PR),
                            xa[co].rearrange("p (b n) -> p b n", n=N)[:, :, 0:PR])
                    else:
                        g = c0
                        while g < c0 + cw:
                            b = g // PC
                            p0 = g % PC
                            run = min(PC - p0, c0 + cw - g)
                            nc.vector.tensor_add(
                                yt[:, g - c0:g - c0 + run],
                                tmp[:, g - c0:g - c0 + run],
                                xa[co][:, N * b + PR + p0:N * b + PR + p0 + run])
                            g += run
                    nc.sync.dma_start(
                        out=d["out"][128 * co:128 * (co + 1), c0:c0 + cw],
                        in_=yt[:, :cw])
        for p in (ps_f2, ps_f1, ypool, w2pool, w1pool, hpool):
            p.close()
    p_xn2.close()
    xa_pool.close()
    const.close()


# --------------------------------------------------------------------------

def make_in_maps(inputs):
    w = prep_weights({k: v for k, v in inputs.items() if k != "x"})
    xs = prep_x(np.asarray(inputs["x"], np.float32))
    return [dict(w, xT=xs[c]) for c in range(NCORES)]


def assemble_out(results):
    """Device output is channel-major group-major [C, TT] per core.
    Host: transpose + un-permute tokens to [B, N, C]."""
    out = np.empty((B, N, C), np.float32)
    for c in range(NCORES):
        y = results[c]["out"]                      # [C, TT]
        ytm = np.ascontiguousarray(y.T)            # [TT, C]
        pc = ytm[:NPC].reshape(BL, PC, C)          # [b, cls+patch, C]
        pr = ytm[NPC:].reshape(BL, PR, C)
        ob = out[c * BL:(c + 1) * BL]
        ob[:, 0:1] = pc[:, 0:1]
        ob[:, 1:33] = pr
        ob[:, 33:] = pc[:, 1:]
    return out


def _kernel_impl(inputs, trace=False):
    nc = bacc.Bacc("TRN2", target_bir_lowering=False, debug=False,
                   num_devices=NCORES)
    build_program(nc)
    nc.compile()
    from concourse.bass_utils import run_bass_kernel_spmd
    res = run_bass_kernel_spmd(nc, make_in_maps(inputs), list(range(NCORES)),
                               trace=trace)
    return assemble_out(res.results).astype(np.float32), res.exec_time_ns


def _kernel_impl_res(inputs, trace=False):
    nc = bacc.Bacc("TRN2", target_bir_lowering=False, debug=False,
                   num_devices=NCORES)
    build_program(nc)
    nc.compile()
    from concourse.bass_utils import run_bass_kernel_spmd
    res = run_bass_kernel_spmd(nc, make_in_maps(inputs), list(range(NCORES)),
                               trace=trace)
    return (assemble_out(res.results).astype(np.float32), res.exec_time_ns,
            res)


def kernel(**inputs):
    return _kernel_impl(inputs, trace=False)[0]



# revision 4
# speedup vs baseline: 1.0583x; 1.0583x over previous
"""VPT-style transformer block kernel for TRN2, 8-core data-parallel. v2.

Token order per batch is permuted to PCP = [prompts(32), cls(1), patch(196)];
attention is permutation-equivariant under a consistent permutation of q/k/v +
mask, so we only un-permute at the final output DMA.

v2 changes vs baseline:
  - all weight DRAM layouts pre-transposed so every SBUF weight-tile DMA is
    one fully-contiguous block (was: 256B-packet strided scatter)
  - softmax 1/z + LN rstd via DVE reciprocal_approx_fast (was ScalarE
    Reciprocal LUT, which table-thrashed against Exp every head)
  - qk evacuation on DVE tensor_scalar_add with 2-batch-aligned chunks
  - LN stats via ones[128,128] matmul (stats pre-broadcast on all partitions),
    LN apply on GpSimd with ct-broadcast APs, per-batch (pipelines into v)
  - s matmuls row-tiled (K=64 head pairs), av/z matmuls col-tiled (M=64 head
    pairs) -> 2x PE concurrency in attention inner loop
  - pr v-groups packed 4 batches per matmul (M=128 instead of 32)
  - MLP runs in 2 pc token-halves + pr: fc1/fc2 weights streamed 2x instead
    of 5x (48MB vs 80MB), psum 2-bank tiles, residual add on GpSimd
  - PE warmup matmuls during the input DMA to lift the HAM clock gate
"""

import numpy as np
import ml_dtypes

import concourse.bass as bass
import concourse.mybir as mybir
import concourse.tile as tile
from concourse import bacc

F32 = mybir.dt.float32
BF16 = mybir.dt.bfloat16
AF = mybir.ActivationFunctionType
ALU = mybir.AluOpType

B, N, C, H, O, P = 64, 229, 1024, 16, 32, 196
D = C // H
SCALE = D ** -0.5
EPS = 1e-5
HID = 4 * C
NCORES = 8
BL = B // NCORES      # 8
PC = 1 + P            # 197
PR = O                # 32
TT = BL * N           # 1832
NPC = BL * PC         # 1576
NPR = BL * PR         # 256
CT = C // 128         # 8
HT = HID // 128       # 32
CTT = CT * TT         # 14656

KC0 = (0, 128)        # PCP tokens 0..127   (pr 0..31 + pc 0..95)
KC1 = (101, 128)      # PCP tokens 101..228 (pc 69..196); rows 0..26 dup-zeroed

LN_CHUNKS = [(0, 512), (512, 512), (1024, 512), (1536, TT - 1536)]


def _bf(x):
    return np.asarray(x, dtype=ml_dtypes.bfloat16)


def prep_weights(i):
    """Host-side: fold LN gains/biases into weights, cast to bf16.
    All 4D weight tensors laid out [out_tile, p, in_tile, e] so that one
    SBUF tile load is a single contiguous 256KB..1MB DMA."""
    i = {k: np.asarray(v, np.float32) for k, v in i.items()}
    w = {}
    for tag, wqkv, bqkv, g, b in (
        ("pc", i["w_qkv"], i["b_qkv"], i["n1_g"], i["n1_b"]),
        ("pr", i["w_qkv_p"], i["b_qkv_p"], i["n1p_g"], i["n1p_b"]),
    ):
        weff = wqkv * g[:, None]
        beff = bqkv + b @ wqkv
        wqk = weff[:, : 2 * C]
        w[f"wqk_{tag}"] = _bf(np.ascontiguousarray(
            wqk.reshape(CT, 128, 16, 128).transpose(2, 1, 0, 3)))
        w[f"bqk_{tag}"] = np.ascontiguousarray(beff[: 2 * C]).astype(np.float32)
        w[f"wv_{tag}"] = _bf(np.ascontiguousarray(weff[:, 2 * C:]))
        w[f"bv_{tag}"] = np.ascontiguousarray(beff[2 * C:]).astype(np.float32)

    for tag, wp, bp in (("pc", i["w_proj"], i["b_proj"]),
                        ("pr", i["w_proj_p"], i["b_proj_p"])):
        w[f"wproj_{tag}"] = _bf(np.ascontiguousarray(
            wp.reshape(CT, 128, CT, 128).transpose(2, 1, 0, 3)))
        w[f"bproj_{tag}"] = np.asarray(bp, np.float32)

    for tag, f1w, f1b, f2w, f2b, g, b in (
        ("pc", i["fc1_w"], i["fc1_b"], i["fc2_w"], i["fc2_b"], i["n2_g"], i["n2_b"]),
        ("pr", i["pfc1_w"], i["pfc1_b"], i["pfc2_w"], i["pfc2_b"], i["n2p_g"], i["n2p_b"]),
    ):
        f1eff = f1w * g[:, None]
        f1beff = f1b + b @ f1w
        w[f"wf1_{tag}"] = _bf(np.ascontiguousarray(
            f1eff.reshape(CT, 128, HT, 128).transpose(2, 1, 0, 3)))
        w[f"bf1_{tag}"] = np.asarray(f1beff, np.float32)
        w[f"wf2_{tag}"] = _bf(np.ascontiguousarray(
            f2w.reshape(HT, 128, CT, 128).transpose(2, 1, 0, 3)))
        w[f"bf2_{tag}"] = np.asarray(f2b, np.float32)
    return w


PERM = np.concatenate([np.arange(1, 33), [0], np.arange(33, 229)])


def prep_x(x):
    xp = x[:, PERM, :]
    xp = xp.reshape(NCORES, BL * N, C)
    return [np.ascontiguousarray(xp[c].T).astype(np.float32) for c in range(NCORES)]


class _Pool:
    def __init__(self, tc, **kw):
        self._cm = tc.tile_pool(**kw)
        self._p = self._cm.__enter__()

    def tile(self, *a, **k):
        if "name" not in k:
            k["name"] = k.get("tag") or "t"
        return self._p.tile(*a, **k)

    def close(self):
        self._cm.__exit__(None, None, None)


def build_program(nc):
    def din(name, shape, dt):
        return nc.dram_tensor(name, list(shape), dt, kind="ExternalInput").ap()

    d = {}
    d["xT"] = din("xT", (C, TT), F32)
    for t in ("pc", "pr"):
        d[f"wqk_{t}"] = din(f"wqk_{t}", (16, 128, CT, 128), BF16)
        d[f"bqk_{t}"] = din(f"bqk_{t}", (2 * C,), F32)
        d[f"wv_{t}"] = din(f"wv_{t}", (C, C), BF16)
        d[f"bv_{t}"] = din(f"bv_{t}", (C,), F32)
        d[f"wproj_{t}"] = din(f"wproj_{t}", (CT, 128, CT, 128), BF16)
        d[f"bproj_{t}"] = din(f"bproj_{t}", (C,), F32)
        d[f"wf1_{t}"] = din(f"wf1_{t}", (HT, 128, CT, 128), BF16)
        d[f"bf1_{t}"] = din(f"bf1_{t}", (HID,), F32)
        d[f"wf2_{t}"] = din(f"wf2_{t}", (CT, 128, HT, 128), BF16)
        d[f"bf2_{t}"] = din(f"bf2_{t}", (C,), F32)
    d["out"] = nc.dram_tensor("out", [C, TT], F32, kind="ExternalOutput").ap()
    with tile.TileContext(nc) as tc:
        _emit(tc, nc, d)


def _psplit2(dst0, src0, nrows):
    cap = {0: 128, 32: 32, 64: 64, 96: 32}
    out = []
    done = 0
    while done < nrows:
        a, b = dst0 + done, src0 + done
        n = min(cap[a], cap[b], nrows - done)
        out.append((a, b, n))
        done += n
    return out


def _ln_chunk(nc, pools, xa, rb, mrb, consts, chunk):
    """One LN stats chunk: sums/sumsq matmuls + row math for those columns."""
    ones128, eps128 = consts
    ps_pool, sq_pool, st_pool = pools
    c0, cw = chunk
    ps_s = ps_pool.tile([128, 512], F32, tag="s")
    ps_q = ps_pool.tile([128, 512], F32, tag="q")
    for ti in range(CT):
        xs = xa[:, TT * ti + c0: TT * ti + c0 + cw]
        xbf = sq_pool.tile([128, 512], BF16, tag="xbf")
        nc.scalar.activation(xbf[:, :cw], xs, AF.Copy)
        xsq = sq_pool.tile([128, 512], BF16, tag="xsq")
        nc.vector.tensor_mul(xsq[:, :cw], xs, xs)
        nc.tensor.matmul(ps_s[:, :cw], ones128, xbf[:, :cw],
                         start=(ti == 0), stop=(ti == CT - 1))
        nc.tensor.matmul(ps_q[:, :cw], ones128, xsq[:, :cw],
                         start=(ti == 0), stop=(ti == CT - 1))
    m_sb = st_pool.tile([128, 512], F32, tag="m")
    nc.scalar.activation(m_sb[:, :cw], ps_s[:, :cw], AF.Copy, scale=1.0 / C)
    t1 = st_pool.tile([128, 512], F32, tag="t1")
    nc.vector.tensor_mul(t1[:, :cw], m_sb[:, :cw], m_sb[:, :cw])
    t2 = st_pool.tile([128, 512], F32, tag="t2")
    nc.vector.scalar_tensor_tensor(t2[:, :cw], ps_q[:, :cw], 1.0 / C,
                                   t1[:, :cw], ALU.mult, ALU.subtract)
    sd = st_pool.tile([128, 512], F32, tag="t1")
    nc.scalar.activation(sd[:, :cw], t2[:, :cw], AF.Sqrt, bias=eps128)
    rinv = st_pool.tile([128, 512], F32, tag="t2")
    nc.vector.reciprocal_approx_fast(rinv[:, :cw], sd[:, :cw])
    nc.vector.tensor_copy(rb[:, c0:c0 + cw], rinv[:, :cw])
    nc.vector.tensor_mul(mrb[:, c0:c0 + cw], m_sb[:, :cw], rinv[:, :cw])


def _ln(nc, tc, xa, rb, mrb, consts, tag):
    """LayerNorm stats over channel (partition) dim of b-major xa big tile.
    Writes rb (rstd) and mrb (mean*rstd) as [128, TT] f32, values broadcast
    across partitions already (stats matmul uses ones [128,128] lhsT)."""
    ones128, eps128 = consts
    ps_pool = _Pool(tc, name=f"lnps_{tag}", bufs=2, space="PSUM")
    sq_pool = _Pool(tc, name=f"lnsq_{tag}", bufs=4)
    st_pool = _Pool(tc, name=f"lnst_{tag}", bufs=2)
    for (c0, cw) in LN_CHUNKS:
        ps_s = ps_pool.tile([128, 512], F32, tag="s")
        ps_q = ps_pool.tile([128, 512], F32, tag="q")
        for ti in range(CT):
            xs = xa[:, TT * ti + c0: TT * ti + c0 + cw]
            xbf = sq_pool.tile([128, 512], BF16, tag="xbf")
            nc.scalar.activation(xbf[:, :cw], xs, AF.Copy)
            xsq = sq_pool.tile([128, 512], BF16, tag="xsq")
            nc.vector.tensor_mul(xsq[:, :cw], xs, xs)
            nc.tensor.matmul(ps_s[:, :cw], ones128, xbf[:, :cw],
                             start=(ti == 0), stop=(ti == CT - 1))
            nc.tensor.matmul(ps_q[:, :cw], ones128, xsq[:, :cw],
                             start=(ti == 0), stop=(ti == CT - 1))
        m_sb = st_pool.tile([128, 512], F32, tag="m")
        nc.scalar.activation(m_sb[:, :cw], ps_s[:, :cw], AF.Copy, scale=1.0 / C)
        t1 = st_pool.tile([128, 512], F32, tag="t1")
        nc.vector.tensor_mul(t1[:, :cw], m_sb[:, :cw], m_sb[:, :cw])
        t2 = st_pool.tile([128, 512], F32, tag="t2")
        nc.vector.scalar_tensor_tensor(t2[:, :cw], ps_q[:, :cw], 1.0 / C,
                                       t1[:, :cw], ALU.mult, ALU.subtract)
        sd = st_pool.tile([128, 512], F32, tag="t1")
        nc.scalar.activation(sd[:, :cw], t2[:, :cw], AF.Sqrt, bias=eps128)
        rinv = st_pool.tile([128, 512], F32, tag="t2")
        nc.vector.reciprocal_approx_fast(rinv[:, :cw], sd[:, :cw])
        nc.vector.tensor_copy(rb[:, c0:c0 + cw], rinv[:, :cw])
        nc.vector.tensor_mul(mrb[:, c0:c0 + cw], m_sb[:, :cw], rinv[:, :cw])
    st_pool.close()
    sq_pool.close()
    ps_pool.close()


def _apply_ln_batch(nc, xa, xn, rb, mrb, b, eng=None):
    """xn pc-block for batch b (all 8 ct) = (x - m) * r."""
    eng = eng or nc.gpsimd
    src = bass.AP.rearrange(xa, "p (t c) -> p t c", t=CT)[:, :, 229 * b + 32:
                                                          229 * b + 229]
    dst = bass.AP.rearrange(xn, "p (t c) -> p t c", t=CT)[:, :, 197 * b:
                                                          197 * (b + 1)]
    rv = rb[:, 229 * b + 32:229 * b + 229].unsqueeze(1).broadcast_to((128, CT, 197))
    mv = mrb[:, 229 * b + 32:229 * b + 229].unsqueeze(1).broadcast_to((128, CT, 197))
    eng.tensor_mul(dst, src, rv)
    eng.tensor_sub(dst, dst, mv)


def _apply_ln_pr(nc, xa, xn, rb, mrb, eng=None):
    """xn pr-block (all batches, all ct)."""
    eng = eng or nc.gpsimd
    src = bass.AP.rearrange(xa, "p (t b n) -> p t b n", t=CT, b=BL)[:, :, :, 0:PR]
    dst_v = bass.AP.rearrange(xn, "p (t c) -> p t c", t=CT)[:, :, NPC:TT]
    dst = dst_v.rearrange("p t (b n) -> p t b n", b=BL)
    rv = bass.AP.rearrange(rb, "p (b n) -> p b n", b=BL)[:, :, 0:PR] \
        .unsqueeze(1).broadcast_to((128, CT, BL, PR))
    mv = bass.AP.rearrange(mrb, "p (b n) -> p b n", b=BL)[:, :, 0:PR] \
        .unsqueeze(1).broadcast_to((128, CT, BL, PR))
    eng.tensor_mul(dst, src, rv)
    eng.tensor_sub(dst, dst, mv)


def _emit(tc, nc, d):
    const = _Pool(tc, name="const", bufs=1)
    ones128 = const.tile([128, 128], BF16, tag="ones128")
    nc.vector.memset(ones128, 1.0)
    onesblk = const.tile([128, 64], BF16, tag="onesblk")
    nc.vector.memset(onesblk, 1.0)
    eps128 = const.tile([128, 1], F32, tag="eps128")
    nc.vector.memset(eps128, EPS)
    warm = const.tile([128, 512], BF16, tag="warm")
    nc.vector.memset(warm, 0.5)
    maskb32 = const.tile([128, 1], F32, tag="maskb32")
    nc.vector.memset(maskb32, 0.0)
    nc.vector.memset(maskb32[0:32, :], -88.0)
    maskb27 = const.tile([128, 1], F32, tag="maskb27")
    nc.vector.memset(maskb27, 0.0)
    nc.vector.memset(maskb27[0:27, :], -88.0)

    # bias tiles
    bqk_sb, bproj_sb, bf1_sb, bf2_sb = {}, {}, {}, {}
    for t in ("pc", "pr"):
        bqk_sb[t] = const.tile([128, 16], F32, tag=f"bqk_{t}")
        nc.sync.dma_start(out=bqk_sb[t],
                          in_=d[f"bqk_{t}"].rearrange("(a p) -> p a", p=128))
        bproj_sb[t] = const.tile([128, CT], F32, tag=f"bproj_{t}")
        nc.sync.dma_start(out=bproj_sb[t],
                          in_=d[f"bproj_{t}"].rearrange("(a p) -> p a", p=128))
        bf1_sb[t] = const.tile([128, HT], F32, tag=f"bf1_{t}")
        nc.sync.dma_start(out=bf1_sb[t],
                          in_=d[f"bf1_{t}"].rearrange("(a p) -> p a", p=128))
        bf2_sb[t] = const.tile([128, CT], F32, tag=f"bf2_{t}")
        nc.sync.dma_start(out=bf2_sb[t],
                          in_=d[f"bf2_{t}"].rearrange("(a p) -> p a", p=128))

    # ---- x arena (b-major, overwritten by x1 in place at proj time) ----
    xa_pool = _Pool(tc, name="xarena", bufs=1)
    xa = xa_pool.tile([128, CTT], F32, tag="xa")

    # warmup matmuls to lift the HAM clock gate while x streams in
    ps_warm = _Pool(tc, name="warm_ps", bufs=1, space="PSUM")
    pw = ps_warm.tile([128, 512], F32, tag="w")
    for _ in range(20):
        nc.tensor.matmul(pw, ones128, warm, start=True, stop=True)
    ps_warm.close()

    # x DMA, chunk-major so LN1 stats can pipeline
    for (c0, cw) in LN_CHUNKS:
        for ti in range(CT):
            nc.sync.dma_start(
                out=xa[:, TT * ti + c0: TT * ti + c0 + cw],
                in_=d["xT"][128 * ti:128 * (ti + 1), c0:c0 + cw])

    # persistent pools, stack-ordered by close time
    # p_xn1 deepest: its tile is reused as xn2 through the MLP
    p_xn1 = _Pool(tc, name="p_xn1", bufs=1)
    xn1 = p_xn1.tile([128, CTT], BF16, tag="xn1")
    p_opr = _Pool(tc, name="p_opr", bufs=1)
    o_pr = [p_opr.tile([128, NPR], BF16, tag=f"opr{i}") for i in range(CT)]
    p_obp = _Pool(tc, name="p_obp", bufs=1)
    obp = [[p_obp.tile([128, 2 * PC], BF16, tag=f"obp{bp}_{i}")
            for i in range(CT)] for bp in range(BL // 2)]
    p_vT = _Pool(tc, name="p_vT", bufs=1)
    vT = [[p_vT.tile([128, 1024], BF16, tag=f"vT{b}_{kc}")
           for kc in range(2)] for b in range(BL)]
    rbm1 = _Pool(tc, name="rbm1", bufs=1)
    rb1 = rbm1.tile([128, TT], F32, tag="rb1")
    mrb1 = rbm1.tile([128, TT], F32, tag="mrb1")

    with nc.named_scope("ln1"):
        _ln(nc, tc, xa, rb1, mrb1, (ones128, eps128), "ln1")

    # ---------------- v (transposed) + LN1 apply pipelined ----------------
    with nc.named_scope("vmm"):
        # bv broadcast tiles [128, C]
        p_bvb = _Pool(tc, name="p_bvb", bufs=1)
        p_row = _Pool(tc, name="p_bvrow", bufs=1)
        ps_bc = _Pool(tc, name="vbc_ps", bufs=2, space="PSUM")
        bvb = {}
        for t in ("pc", "pr"):
            brow = p_row.tile([1, C], F32, tag=f"bvr_{t}")
            nc.sync.dma_start(out=brow,
                              in_=d[f"bv_{t}"].rearrange("(o c) -> o c", o=1))
            brow_bf = p_row.tile([1, C], BF16, tag=f"bvrb_{t}")
            nc.vector.tensor_copy(brow_bf, brow)
            bvb[t] = p_bvb.tile([128, C], F32, tag=f"bvb_{t}")
            for half in range(2):
                pb = ps_bc.tile([128, 512], F32, tag="bc")
                nc.tensor.matmul(pb, ones128[0:1, :],
                                 brow_bf[:, 512 * half:512 * (half + 1)],
                                 start=True, stop=True)
                nc.vector.tensor_copy(bvb[t][:, 512 * half:512 * (half + 1)], pb)
        ps_bc.close()
        p_row.close()

        p_wv = _Pool(tc, name="p_wv", bufs=1)
        ps_v = _Pool(tc, name="v_ps", bufs=2, space="PSUM")

        # pr first: apply pr + packed pr v-groups (4 batches per matmul)
        _apply_ln_pr(nc, xa, xn1, rb1, mrb1)
        wv_pr = [p_wv.tile([128, C], BF16, tag=f"wvpr{ci}") for ci in range(CT)]
        for ci in range(CT):
            nc.sync.dma_start(out=wv_pr[ci],
                              in_=d["wv_pr"][128 * ci:128 * (ci + 1), :])
        for g in range(2):
            pv = ps_v.tile([128, 1024], F32, tag="v")
            for ti in range(CT):
                lhs = xn1[:, TT * ti + NPC + 128 * g: TT * ti + NPC + 128 * (g + 1)]
                for half in range(2):
                    nc.tensor.matmul(pv[:, 512 * half:512 * (half + 1)], lhs,
                                     wv_pr[ti][:, 512 * half:512 * (half + 1)],
                                     start=(ti == 0), stop=(ti == CT - 1))
            for j in range(4):
                nc.vector.tensor_add(vT[4 * g + j][0][0:32, :],
                                     pv[32 * j:32 * j + 32, :],
                                     bvb["pr"][32 * j:32 * j + 32, :])

        # pc: apply per batch, then that batch's two v-groups
        wv_pc = [p_wv.tile([128, C], BF16, tag=f"wvpc{ci}") for ci in range(CT)]
        for ci in range(CT):
            nc.sync.dma_start(out=wv_pc[ci],
                              in_=d["wv_pc"][128 * ci:128 * (ci + 1), :])
        for b in range(BL):
            _apply_ln_batch(nc, xa, xn1, rb1, mrb1, b)
            for kc, off, nrows, dst0 in ((0, 0, 96, 32), (1, 69, 128, 0)):
                pv = ps_v.tile([128, 1024], F32, tag="v")
                for ti in range(CT):
                    lhs = xn1[:, TT * ti + 197 * b + off:
                              TT * ti + 197 * b + off + nrows]
                    for half in range(2):
                        nc.tensor.matmul(
                            pv[:nrows, 512 * half:512 * (half + 1)], lhs,
                            wv_pc[ti][:, 512 * half:512 * (half + 1)],
                            start=(ti == 0), stop=(ti == CT - 1))
                for d0, s0, sn in _psplit2(dst0, 0, nrows):
                    nc.vector.tensor_add(vT[b][kc][d0:d0 + sn, :],
                                         pv[s0:s0 + sn, :],
                                         bvb["pc"][d0:d0 + sn, :])
        ps_v.close()
        p_wv.close()
        p_bvb.close()
    rbm1.close()

    # ---- attention: qk chunk-groups interleaved into softmax gaps ----
    qk_pool = _Pool(tc, name="qk", bufs=1)
    wq_pool = _Pool(tc, name="wqk", bufs=2)

    with nc.named_scope("attn"):
        ps_qk = _Pool(tc, name="qk_ps", bufs=2, space="PSUM")
        se_pool = _Pool(tc, name="se_ps", bufs=1, space="PSUM")
        poz_pool = _Pool(tc, name="poz_ps", bufs=1, space="PSUM")
        es_pool = _Pool(tc, name="es", bufs=2)
        zb_pool = _Pool(tc, name="zb", bufs=2)
        qk_t = {}

        def emit_qk_group(hp, kind, cc):
            co = hp if kind == "q" else 8 + hp
            if cc == 0:
                w_sb = {}
                for t in ("pc", "pr"):
                    w_sb[t] = wq_pool.tile([128, CT * 128], BF16,
                                           tag=f"w_{kind}_{t}")
                    nc.sync.dma_start(
                        out=w_sb[t],
                        in_=d[f"wqk_{t}"][co].rearrange("p c e -> p (c e)"))
                qk_t[(hp, kind, "w")] = w_sb
                qk_t[(hp, kind)] = qk_pool.tile([128, TT], BF16,
                                                tag=f"{kind}{hp % 2}")
            w_sb = qk_t[(hp, kind, "w")]
            qt = qk_t[(hp, kind)]
            pq = ps_qk.tile([128, 512], F32, tag="qk")
            if cc < 4:
                for ti in range(CT):
                    nc.tensor.matmul(
                        pq[:, :394], w_sb["pc"][:, 128 * ti:128 * (ti + 1)],
                        xn1[:, TT * ti + 394 * cc: TT * ti + 394 * (cc + 1)],
                        start=(ti == 0), stop=(ti == CT - 1))
                dst = bass.AP.rearrange(qt, "p (b n) -> p b n", n=N)[
                    :, 2 * cc:2 * cc + 2, PR:N]
                nc.vector.tensor_scalar_add(
                    dst, pq[:, :394].rearrange("p (b n) -> p b n", b=2),
                    bqk_sb["pc"][:, co:co + 1])
            else:
                for ti in range(CT):
                    nc.tensor.matmul(
                        pq[:, :256], w_sb["pr"][:, 128 * ti:128 * (ti + 1)],
                        xn1[:, TT * ti + NPC: TT * ti + NPC + NPR],
                        start=(ti == 0), stop=(ti == CT - 1))
                dst = bass.AP.rearrange(qt, "p (b n) -> p b n", n=N)[:, :, 0:PR]
                nc.vector.tensor_scalar_add(
                    dst, pq[:, :256].rearrange("p (b n) -> p b n", b=BL),
                    bqk_sb["pr"][:, co:co + 1])

        for kind in ("q", "k"):
            for cc in range(5):
                emit_qk_group(0, kind, cc)

        GAP_PLAN = [3, 3, 2, 2]
        for hp in range(8):
            qt_q = qk_t[(hp, "q")]
            qt_k = qk_t[(hp, "k")]
            nxt = ([(hp + 1, kind, cc) for kind in ("q", "k")
                    for cc in range(5)] if hp < 7 else [])
            gi = 0
            for bp in range(BL // 2):
                b0 = 2 * bp
                se = se_pool.tile([128, 2048], F32, tag="se")
                for kc, (t0, tw) in enumerate((KC0, KC1)):
                    for j in range(2):
                        for hh in range(2):
                            r0 = 64 * hh
                            nc.tensor.matmul(
                                se[:tw, 1024 * hh + 512 * kc + N * j:
                                   1024 * hh + 512 * kc + N * j + N],
                                qt_k[r0:r0 + 64,
                                     N * (b0 + j) + t0: N * (b0 + j) + t0 + tw],
                                qt_q[r0:r0 + 64, N * (b0 + j):N * (b0 + j + 1)],
                                start=True, stop=True)
                # next-hp qk groups fill the PE gap while ScalarE runs exp
                for _ in range(GAP_PLAN[bp]):
                    if gi < len(nxt):
                        emit_qk_group(*nxt[gi])
                        gi += 1
                e = es_pool.tile([128, 1832], BF16, tag="e")
                e4 = bass.AP.rearrange(e, "p (h r) -> p h r", h=2)[
                    :, :, 0:458].rearrange("p h (j n) -> p h j n", j=2)
                s4 = bass.AP.rearrange(se, "p (h r) -> p h r", h=2)[
                    :, :, 0:458].rearrange("p h (j n) -> p h j n", j=2)
                # kc0 pc q-cols (both heads, both batches): prompt rows masked
                nc.scalar.activation(e4[:, :, :, PR:N], s4[:, :, :, PR:N],
                                     AF.Exp, scale=SCALE, bias=maskb32)
                # kc0 pr q-cols: unmasked
                nc.scalar.activation(e4[:, :, :, 0:PR], s4[:, :, :, 0:PR],
                                     AF.Exp, scale=SCALE)
                # kc1 (both heads): duplicate k-rows 0..26 masked
                nc.scalar.activation(
                    bass.AP.rearrange(e, "p (h r) -> p h r", h=2)[:, :, 458:916],
                    bass.AP.rearrange(se, "p (h r) -> p h r", h=2)[:, :, 512:970],
                    AF.Exp, scale=SCALE, bias=maskb27)
                poz = poz_pool.tile([128, 1024], F32, tag="poz")
                for j in range(2):
                    for kc in range(2):
                        for hh in range(2):
                            h_abs = 2 * hp + hh
                            eap = e[:, 916 * hh + 458 * kc + N * j:
                                    916 * hh + 458 * kc + N * (j + 1)]
                            nc.tensor.matmul(
                                poz[64 * hh:64 * hh + 64, N * j:N * (j + 1)],
                                vT[b0 + j][kc][:, 64 * h_abs:64 * h_abs + 64],
                                eap, start=(kc == 0), stop=(kc == 1))
                            nc.tensor.matmul(
                                poz[64 * hh:64 * hh + 64,
                                    512 + N * j:512 + N * (j + 1)],
                                onesblk, eap,
                                start=(kc == 0), stop=(kc == 1))
                zb = zb_pool.tile([128, 458], F32, tag="zb")
                nc.vector.reciprocal_approx_fast(zb, poz[:, 512:970])
                po_v = bass.AP.rearrange(poz[:, 0:458], "p (b n) -> p b n", b=2)
                zb_v = bass.AP.rearrange(zb, "p (b n) -> p b n", b=2)
                nc.vector.tensor_mul(
                    bass.AP.rearrange(obp[bp][hp], "p (b n) -> p b n", b=2),
                    po_v[:, :, PR:N], zb_v[:, :, PR:N])
                nc.vector.tensor_mul(
                    bass.AP.rearrange(o_pr[hp][:, PR * b0:PR * (b0 + 2)],
                                      "p (b n) -> p b n", b=2),
                    po_v[:, :, 0:PR], zb_v[:, :, 0:PR])
            while gi < len(nxt):
                emit_qk_group(*nxt[gi])
                gi += 1
        zb_pool.close()
        es_pool.close()
        poz_pool.close()
        se_pool.close()
        ps_qk.close()
    wq_pool.close()
    qk_pool.close()

    # ------- proj (+residual into xa) with LN2 stats interleaved -------
    rbm2 = _Pool(tc, name="rbm2", bufs=1)
    rb2 = rbm2.tile([128, TT], BF16, tag="rb2")
    mrb2 = rbm2.tile([128, TT], BF16, tag="mrb2")
    ln2_ps = _Pool(tc, name="lnps_ln2", bufs=2, space="PSUM")
    ln2_sq = _Pool(tc, name="lnsq_ln2", bufs=4)
    ln2_st = _Pool(tc, name="lnst_ln2", bufs=2)
    ln2_pools = (ln2_ps, ln2_sq, ln2_st)
    lnc = (ones128, eps128)
    xn2 = xn1  # reuse the attention-normalized arena (dead after attn)

    with nc.named_scope("proj"):
        wp_pool = _Pool(tc, name="wproj", bufs=2)
        pj_tmp = _Pool(tc, name="pj_tmp", bufs=2)
        ps_pj = _Pool(tc, name="pj_ps", bufs=2, space="PSUM")
        for co in range(CT):
            w_sb = wp_pool.tile([128, CT * 128], BF16, tag="w")
            nc.sync.dma_start(
                out=w_sb, in_=d["wproj_pr"][co].rearrange("p c e -> p (c e)"))
            pt = ps_pj.tile([128, NPR], F32, tag="pj")
            for ci in range(CT):
                nc.tensor.matmul(pt, w_sb[:, 128 * ci:128 * (ci + 1)], o_pr[ci],
                                 start=(ci == 0), stop=(ci == CT - 1))
            tmp = pj_tmp.tile([128, NPR], F32, tag="t")
            nc.vector.tensor_scalar_add(tmp, pt, bproj_sb["pr"][:, co:co + 1])
            xap = xa[:, TT * co: TT * (co + 1)].rearrange(
                "p (b n) -> p b n", n=N)[:, :, 0:PR]
            nc.vector.tensor_add(
                xap, bass.AP.rearrange(tmp, "p (b n) -> p b n", b=BL), xap)
        for bp in range(BL // 2):
            for co in range(CT):
                w_sb = wp_pool.tile([128, CT * 128], BF16, tag="w")
                nc.sync.dma_start(
                    out=w_sb, in_=d["wproj_pc"][co].rearrange("p c e -> p (c e)"))
                pt = ps_pj.tile([128, 2 * PC], F32, tag="pj")
                for ci in range(CT):
                    nc.tensor.matmul(pt, w_sb[:, 128 * ci:128 * (ci + 1)],
                                     obp[bp][ci],
                                     start=(ci == 0), stop=(ci == CT - 1))
                tmp = pj_tmp.tile([128, 2 * PC], F32, tag="t")
                nc.vector.tensor_scalar_add(tmp, pt, bproj_sb["pc"][:, co:co + 1])
                xap = xa[:, TT * co: TT * (co + 1)].rearrange(
                    "p (b n) -> p b n", n=N)[:, 2 * bp:2 * bp + 2, PR:N]
                nc.vector.tensor_add(
                    xap, bass.AP.rearrange(tmp, "p (b n) -> p b n", b=2), xap)
            if bp == 2:
                # x1 cols 0:1024 (batches 0-5) are final -> first two LN2
                # stats chunks overlap the remaining proj work
                with nc.named_scope("ln2"):
                    _ln_chunk(nc, ln2_pools, xa, rb2, mrb2, lnc, LN_CHUNKS[0])
                    _ln_chunk(nc, ln2_pools, xa, rb2, mrb2, lnc, LN_CHUNKS[1])
        ps_pj.close(); pj_tmp.close(); wp_pool.close()

    # ---------------- LN2 tail + apply ----------------
    with nc.named_scope("ln2"):
        _ln_chunk(nc, ln2_pools, xa, rb2, mrb2, lnc, LN_CHUNKS[2])
        _ln_chunk(nc, ln2_pools, xa, rb2, mrb2, lnc, LN_CHUNKS[3])
        ln2_st.close(); ln2_sq.close(); ln2_ps.close()
        for b in range(BL):
            eng = nc.vector if b % 2 == 0 else nc.gpsimd
            _apply_ln_batch(nc, xa, xn2, rb2, mrb2, b, eng=eng)
        _apply_ln_pr(nc, xa, xn2, rb2, mrb2)
    rbm2.close()
    p_vT.close()
    p_obp.close()
    p_opr.close()

    # ---------------- MLP + output ----------------
    with nc.named_scope("mlp"):
        hpool = _Pool(tc, name="h", bufs=1)
        w1pool = _Pool(tc, name="wf1", bufs=3)
        w2pool = _Pool(tc, name="wf2", bufs=3)
        ypool = _Pool(tc, name="y", bufs=2)
        ps_f1 = _Pool(tc, name="f1_ps", bufs=2, space="PSUM")
        ps_f2 = _Pool(tc, name="f2_ps", bufs=2, space="PSUM")

        # parts: (tag, col0, ncols, nsub, subw)
        parts = [("pc", 0, 788, 2, 394), ("pc", 788, 788, 2, 394),
                 ("pr", NPC, 256, 1, 256)]
        for (t, p0, ncols, nsub, subw) in parts:
            hs = []
            for hc in range(HT):
                w1 = w1pool.tile([128, CT * 128], BF16, tag="w1")
                nc.sync.dma_start(
                    out=w1, in_=d[f"wf1_{t}"][hc].rearrange("p c e -> p (c e)"))
                ph = ps_f1.tile([128, 1024], F32, tag="f1")
                for s in range(nsub):
                    for ti in range(CT):
                        nc.tensor.matmul(
                            ph[:, 512 * s:512 * s + subw],
                            w1[:, 128 * ti:128 * (ti + 1)],
                            xn2[:, TT * ti + p0 + subw * s:
                                TT * ti + p0 + subw * (s + 1)],
                            start=(ti == 0), stop=(ti == CT - 1))
                hsb = hpool.tile([128, 788], BF16, tag=f"h{hc}")
                if nsub == 2:
                    nc.scalar.activation(
                        bass.AP.rearrange(hsb, "p (s n) -> p s n", s=2),
                        bass.AP.rearrange(ph, "p (s n) -> p s n", s=2)[:, :, 0:394],
                        AF.Gelu, bias=bf1_sb[t][:, hc:hc + 1])
                else:
                    nc.scalar.activation(hsb[:, :256], ph[:, :256],
                                         AF.Gelu, bias=bf1_sb[t][:, hc:hc + 1])
                hs.append(hsb)
            for co in range(CT):
                w2 = w2pool.tile([128, HT * 128], BF16, tag="w2")
                nc.sync.dma_start(
                    out=w2, in_=d[f"wf2_{t}"][co].rearrange("p c e -> p (c e)"))
                py = ps_f2.tile([128, 1024], F32, tag="f2")
                for hc in range(HT):
                    for s in range(nsub):
                        nc.tensor.matmul(
                            py[:, 512 * s:512 * s + subw],
                            w2[:, 128 * hc:128 * (hc + 1)],
                            hs[hc][:, subw * s:subw * (s + 1)],
                            start=(hc == 0), stop=(hc == HT - 1))
                tmp = ypool.tile([128, 788], F32, tag="f2t")
                yt = ypool.tile([128, 788], F32, tag="y")
                if t == "pc":
                    nc.vector.tensor_scalar_add(
                        bass.AP.rearrange(tmp, "p (s n) -> p s n", s=2),
                        bass.AP.rearrange(py, "p (s n) -> p s n", s=2)[:, :, 0:394],
                        bf2_sb[t][:, co:co + 1])
                    b0 = 4 * (p0 // 788)
                    xap = xa[:, TT * co: TT * (co + 1)].rearrange(
                        "p (b n) -> p b n", n=N)[:, b0:b0 + 4, PR:N]
                    nc.vector.tensor_add(
                        bass.AP.rearrange(yt, "p (b n) -> p b n", n=197),
                        bass.AP.rearrange(tmp, "p (b n) -> p b n", n=197), xap)
                    nc.sync.dma_start(
                        out=d["out"][128 * co:128 * (co + 1), p0:p0 + 788],
                        in_=yt)
                else:
                    nc.vector.tensor_scalar_add(tmp[:, :256], py[:, :256],
                                                bf2_sb[t][:, co:co + 1])
                    xap = xa[:, TT * co: TT * (co + 1)].rearrange(
                        "p (b n) -> p b n", n=N)[:, :, 0:PR]
                    nc.vector.tensor_add(
                        bass.AP.rearrange(yt[:, :256], "p (b n) -> p b n", b=BL),
                        bass.AP.rearrange(tmp[:, :256], "p (b n) -> p b n", b=BL),
                        xap)
                    nc.sync.dma_start(
                        out=d["out"][128 * co:128 * (co + 1), NPC:TT],
                        in_=yt[:, :256])
        for p in (ps_f2, ps_f1, ypool, w2pool, w1pool, hpool):
            p.close()
    p_xn1.close()
    xa_pool.close()
    const.close()


# --------------------------------------------------------------------------

def make_in_maps(inputs):
    w = prep_weights({k: v for k, v in inputs.items() if k != "x"})
    xs = prep_x(np.asarray(inputs["x"], np.float32))
    return [dict(w, xT=xs[c]) for c in range(NCORES)]


def assemble_out(results):
    out = np.empty((B, N, C), np.float32)
    for c in range(NCORES):
        y = results[c]["out"]                      # [C, TT] group-major
        ytm = np.ascontiguousarray(y.T)            # [TT, C]
        pc = ytm[:NPC].reshape(BL, PC, C)
        pr = ytm[NPC:].reshape(BL, PR, C)
        ob = out[c * BL:(c + 1) * BL]
        ob[:, 0:1] = pc[:, 0:1]
        ob[:, 1:33] = pr
        ob[:, 33:] = pc[:, 1:]
    return out


def _kernel_impl_res(inputs, trace=False):
    nc = bacc.Bacc("TRN2", target_bir_lowering=False, debug=False,
                   num_devices=NCORES)
    build_program(nc)
    nc.compile()
    from concourse.bass_utils import run_bass_kernel_spmd
    res = run_bass_kernel_spmd(nc, make_in_maps(inputs), list(range(NCORES)),
                               trace=trace)
    return (assemble_out(res.results).astype(np.float32), res.exec_time_ns, res)


def _kernel_impl(inputs, trace=False):
    out, ns, _ = _kernel_impl_res(inputs, trace)
    return out, ns


def kernel(**inputs):
    return _kernel_impl(inputs, trace=False)[0]
